# revision 39
# baseline (speedup 1.0000x reference)
"""CropAndResize (TF-style, crop 14x14) on 8 Trainium2 NeuronCores.

Strategy (data-parallel over ROIs, ~125 boxes per core):
  - Host: build a bf16 row-pair image per batch entry in channel-last
    layout: pairs[h, w] = (img[h, w, :], img[h+1, w, :]) -> [H-1, W, 2C].
    With this layout the 4 bilinear corners of one output pixel (rows
    ti/ti+1, cols xs/xs+1) are ONE contiguous 4C read (2 KB bf16), so a
    single SWDGE gather descriptor fetches a whole pixel's inputs.
  - Host: balance boxes across the 8 cores (each core = its own image
    plus at most one "secondary" donor image, concatenated in DRAM so
    int16 gather indices cover both), and precompute per-pixel corner
    weights w = [w_tl, w_bl, w_tr, w_br] * valid in f32.
  - Device: pipeline over units of 2 x 512-pixel gathers: per 128-pixel
    slot the weighted 4-corner sum runs on one of two paths chosen to
    balance the scalar(ACT) and vector(DVE) engines:
      path A (ACT-heavy): 4 ACT muls m_k = w_k * corner_k, then
        unit-batched DVE partial adds into the X/Y halves.
      path D (DVE-only): custom DVE op SCALE2_ADD computes
        x = w0*tl + w1*bl and y = w2*tr + w3*br (2 per-partition
        scalars, 2 tensor streams, one instruction each).
    ONE final TT per unit adds the X and Y halves for all 8 slots.
  - Host: scatter per-core outputs back to original box order, upcast.
"""

import numpy as np
import ml_dtypes

import concourse.bacc as bacc
import concourse.bass as bass
import concourse.tile as tile
from concourse import mybir, library_config, bass_utils

H, W, C = 100, 152, 256
CROP = 14
PX = CROP * CROP          # 196 pixels per box
P = 128                   # SBUF partitions
NCORES = 8
NPIX2 = (H - 1) * W       # 15048 pair-columns per image
C2 = 2 * C                # elems per pair-column (bf16)
C4 = 4 * C                # gather element: 2 adjacent pair-columns
GU = 512                  # indices per dma_gather instruction
SG = GU // P              # slots per gather group (4)
UG = 2                    # gather groups per compute unit
SU = UG * SG              # slots per compute unit (8)

F32 = mybir.dt.float32
BF16 = mybir.dt.bfloat16
I16 = mybir.dt.int16
ADD = mybir.AluOpType.add

_cache = {}
LAST_EXEC_NS = None

_S2A = None


def _register_s2a():
    """Runtime-register the custom DVE op out = in0*s0 + in1*s1."""
    global _S2A
    if _S2A is not None:
        return _S2A
    from concourse import dve_ops
    from concourse.dve_spec import Spec, Src0, Src1, C0, C1, lower
    from concourse.dve_uop import DveOpSpec
    from concourse.dve_table_gen import dve_ver_for

    name = "SCALE2_ADD_CR"
    for op in dve_ops.OPS:
        if op.name == name:
            _S2A = op
            return op
    ver = dve_ver_for("TRN2")
    spec = Spec(
        body=Src0 * C0 + Src1 * C1,
        reference=lambda in0, in1, s0, s1, imm2: (
            in0.astype(np.float32) * s0 + in1 * s1
        ),
    )
    dve_ops._SUB_OPCODE_FOR_NAME[name] = (
        max(dve_ops._SUB_OPCODE_FOR_NAME.values()) + 1
    )
    sha = DveOpSpec(
        name=name,
        opcode=dve_ops.get_dve_sub_opcode(name),
        uops=lower(spec, ver=ver),
        rd1_en=True,
    ).sha(ver)
    op = dve_ops.DveOp(name, spec, subdim=False, uops_sha={ver: sha})
    dve_ops.OPS.append(op)
    dve_ops.CUSTOM_DVE_SPECS[name] = spec
    _S2A = op
    return op


def _grid_params(boxes):
    """f32 mirror of the reference sampling grid -> effective pair-row
    start ts, pair-col start xs, and the 4 corner weights (valid-masked)."""
    f = np.float32
    y1, x1, y2, x2 = boxes[:, 0], boxes[:, 1], boxes[:, 2], boxes[:, 3]
    h_scale = (y2 - y1) * f(H - 1) / f(CROP - 1)
    w_scale = (x2 - x1) * f(W - 1) / f(CROP - 1)
    ar = np.arange(CROP, dtype=np.float32)
    in_y = y1[:, None] * f(H - 1) + ar[None, :] * h_scale[:, None]
    in_x = x1[:, None] * f(W - 1) + ar[None, :] * w_scale[:, None]
    valid_y = (in_y >= 0) & (in_y <= H - 1)
    valid_x = (in_x >= 0) & (in_x <= W - 1)
    top = np.floor(in_y)
    left = np.floor(in_x)
    y_lerp = (in_y - top).astype(np.float32)
    x_lerp = (in_x - left).astype(np.float32)
    ti = np.clip(top, 0, H - 1).astype(np.int32)
    bi = np.clip(top + 1, 0, H - 1).astype(np.int32)
    li = np.clip(left, 0, W - 1).astype(np.int32)
    ri = np.clip(left + 1, 0, W - 1).astype(np.int32)
    # pair-row start + effective lerps (ti==bi -> bottom row of the pair;
    # li==ri -> right col of the pair; invalid pixels are masked anyway)
    ts = np.minimum(ti, H - 2).astype(np.int32)
    yw = np.where(ti == bi, np.float32(1.0), y_lerp).astype(np.float32)
    xs = np.minimum(li, W - 2).astype(np.int32)
    xw = np.where(li == ri, np.float32(1.0), x_lerp).astype(np.float32)

    n = boxes.shape[0]
    yw2 = yw[:, :, None]
    xw2 = xw[:, None, :]
    vm = (valid_y[:, :, None] & valid_x[:, None, :]).astype(np.float32)
    w4 = np.empty((n, CROP, CROP, 4), np.float32)
    w4[..., 0] = (1 - yw2) * (1 - xw2) * vm   # tl
    w4[..., 1] = yw2 * (1 - xw2) * vm         # bl
    w4[..., 2] = (1 - yw2) * xw2 * vm         # tr
    w4[..., 3] = yw2 * xw2 * vm               # br
    idx = (ts[:, :, None] * W + xs[:, None, :]).reshape(n, PX)
    return idx, w4.reshape(n, PX, 4)


def _assign(box_ind):
    """Balance boxes so each core hosts <= L boxes from <= 2 images
    (its own + one donor). Returns per-core global box-index lists,
    per-core secondary image id, and L."""
    n = len(box_ind)
    counts = np.bincount(box_ind, minlength=NCORES)
    lists = [list(np.where(box_ind == k)[0]) for k in range(NCORES)]
    assign = {}
    L = int(counts.max())
    for L in range(-(-n // NCORES), int(counts.max()) + 1):
        donors = {k: int(counts[k]) - L for k in range(NCORES) if counts[k] > L}
        rooms = {k: L - int(counts[k]) for k in range(NCORES) if counts[k] < L}
        trial = {}
        ok = True
        for d, shed in sorted(donors.items(), key=lambda x: -x[1]):
            rem = shed
            for r, room in sorted(rooms.items(), key=lambda x: -x[1]):
                if rem <= 0:
                    break
                take = min(room, rem)
                if take > 0:
                    trial[r] = (d, take)
                    rem -= take
            for r in trial:
                rooms.pop(r, None)
            if rem > 0:
                ok = False
                break
        if ok:
            assign = trial
            break
    pulled = {}
    by_donor = {}
    for r, (d, take) in assign.items():
        by_donor.setdefault(d, []).append((r, take))
    for d, rts in by_donor.items():
        tail = lists[d]
        pos = len(tail)
        for r, take in rts:
            pulled[r] = tail[pos - take: pos]
            pos -= take
        lists[d] = tail[:pos]
    core_boxes, core_sec = [], []
    for k in range(NCORES):
        extra = pulled.get(k, [])
        sec = int(box_ind[extra[0]]) if extra else k
        core_boxes.append(list(lists[k]) + list(extra))
        core_sec.append(sec)
    return core_boxes, core_sec, L


def _build_core_inputs(boxes_k, sel_k, ng):
    """Gather indices (wrapped int16) + per-slot corner weights for one
    core. boxes_k: [m, 4]; sel_k: [m] in {0,1} (primary/secondary)."""
    ntot = ng * GU
    m = boxes_k.shape[0]
    idx, w4 = _grid_params(boxes_k)                      # [m,196], [m,196,4]
    idx = idx + (np.asarray(sel_k, np.int64)[:, None] * NPIX2)
    idx_flat = np.zeros(ntot, np.int16)
    idx_flat[: m * PX] = idx.reshape(-1).astype(np.int16)
    w_flat = np.zeros((ntot, 4), np.float32)
    w_flat[: m * PX] = w4.reshape(-1, 4)
    # wrapped idx layout: idx k at (k%16, k//16), tiled to 128 partitions
    wrapped = idx_flat.reshape(ntot // 16, 16).T         # [16, ntot//16]
    idx_w = np.tile(wrapped, (P // 16, 1))               # [128, ntot//16]
    # weights per (partition, slot, corner), f32
    w_ps = w_flat.reshape(ntot // P, P, 4).transpose(1, 0, 2)  # [P, slots, 4]
    w_all = np.ascontiguousarray(w_ps.reshape(P, -1))
    return idx_w, w_all


def _na_of(u):
    # alternate 3,3,4 ACT-path slots per unit (avg 10/24 of pixels on
    # the ACT path -- measured best balance of the two engines)
    return 4 if u % 3 == 2 else 3


def _build_program(ng):
    s2a = _register_s2a()
    nu = ng // UG
    nc = bacc.Bacc("TRN2", target_bir_lowering=False, debug=False,
                   num_devices=NCORES, num_swdge_queues=4)
    img = nc.dram_tensor("img", [2 * NPIX2 * C2], BF16, kind="ExternalInput")
    idx = nc.dram_tensor("idx", [P, ng * (GU // 16)], I16, kind="ExternalInput")
    wts = nc.dram_tensor("wts", [P, ng * SG * 4], F32, kind="ExternalInput")
    out = nc.dram_tensor("out", [ng * GU * C], BF16, kind="ExternalOutput")

    # overlapping gather view: index unit = one pair-column (C2 bf16),
    # payload = 2 adjacent pair-columns (C4 bf16 = 2 KB)
    gather_src = bass.AP(img, 0, [(C2, 2 * NPIX2 - 1), (1, C4)])

    with tile.TileContext(nc) as tc:
        with (
            tc.tile_pool(name="gat", bufs=4) as gat_pool,
            tc.tile_pool(name="osb", bufs=4) as out_pool,
            tc.tile_pool(name="meta", bufs=1) as meta_pool,
            tc.tile_pool(name="tmp", bufs=3) as tmp_pool,
        ):
            nc.gpsimd.load_library(library_config.mlp)
            idx_t = meta_pool.tile([P, ng * (GU // 16)], I16, tag="idx")
            nc.sync.dma_start(idx_t[:], idx[:])
            w_t = meta_pool.tile([P, ng * SG * 4], F32, tag="wts")
            nc.sync.dma_start(w_t[:], wts[:])

            LOOK = 2  # prefetch distance in units (2 gathers each)
            gts = {}

            def issue_gathers(u):
                gt = gat_pool.tile([P, UG, SG, C4], BF16, tag="g")
                for j in range(UG):
                    g = u * UG + j
                    nc.gpsimd.dma_gather(
                        gt[:, j], gather_src,
                        idx_t[:, g * (GU // 16): (g + 1) * (GU // 16)],
                        GU, GU, C4, elem_step=C2, queue_num=g % 4)
                gts[u] = gt

            for u in range(min(LOOK, nu)):
                issue_gathers(u)
            for u in range(nu):
                if u + LOOK < nu:
                    issue_gathers(u + LOOK)
                gt4 = gts.pop(u)

                def gs(s, lo, hi):
                    return gt4[:, s // SG, s % SG, lo:hi]

                NA = _na_of(u)
                ND = SU - NA
                ot = out_pool.tile([P, SU, C], BF16, tag="o")
                # X/Y halves: path A writes (m0+m1) into X, (m2+m3) into
                # Y; path D writes x/y directly; ONE final TT adds X+Y.
                xt = tmp_pool.tile([P, SU, C], BF16, tag="x")
                yt = tmp_pool.tile([P, SU, C], BF16, tag="y")

                # path A (ACT muls + batched partial adds): slots [0, NA)
                mt = [tmp_pool.tile([P, 4, C], BF16, tag=f"m{k}",
                                    name=f"mt{k}_{u}")
                      for k in range(4)]
                for a in range(NA):
                    sl = u * SU + a
                    for k in range(4):
                        nc.scalar.mul(
                            mt[k][:, a, :], gs(a, k * C, (k + 1) * C),
                            w_t[:, sl * 4 + k: sl * 4 + k + 1])
                nc.vector.tensor_tensor(xt[:, 0:NA, :], mt[0][:, 0:NA, :],
                                        mt[1][:, 0:NA, :], ADD)
                nc.vector.tensor_tensor(yt[:, 0:NA, :], mt[2][:, 0:NA, :],
                                        mt[3][:, 0:NA, :], ADD)

                # path D (custom dual-scale-add on DVE): slots [NA, SU)
                for d in range(ND):
                    s = NA + d
                    sl = u * SU + s
                    kb = sl * 4
                    nc.vector._custom_dve(
                        s2a, out=xt[:, s, :],
                        in0=gs(s, 0, C), in1=gs(s, C, 2 * C),
                        s0=w_t[:, kb: kb + 1], s1=w_t[:, kb + 1: kb + 2])
                    nc.vector._custom_dve(
                        s2a, out=yt[:, s, :],
                        in0=gs(s, 2 * C, 3 * C), in1=gs(s, 3 * C, 4 * C),
                        s0=w_t[:, kb + 2: kb + 3], s1=w_t[:, kb + 3: kb + 4])
                nc.vector.tensor_tensor(ot[:], xt[:], yt[:], ADD)

                # p-major store: each partition writes one contiguous
                # SU*C run; host unscrambles to pixel order
                out_ap = bass.AP(out, u * UG * GU * C,
                                 [(SU * C, P), (C, SU), (1, C)])
                nc.sync.dma_start(out_ap, ot[:])

    nc.compile()
    return nc


def kernel(image, boxes, box_ind):
    image = np.asarray(image, dtype=np.float32)
    boxes = np.asarray(boxes, dtype=np.float32)
    box_ind = np.asarray(box_ind)
    n_boxes = boxes.shape[0]

    core_boxes, core_sec, m_pad = _assign(box_ind)
    ng = -(-(m_pad * PX) // GU)
    ng = -(-ng // UG) * UG

    # channel-last bf16 row-pair images: pairs[k][h*W+w] = img[h,w,:]+img[h+1,w,:]
    image_t = image.transpose(0, 2, 3, 1).astype(ml_dtypes.bfloat16)  # [B,H,W,C]
    pairs = np.concatenate([image_t[:, :-1], image_t[:, 1:]], axis=-1)
    pairs = pairs.reshape(NCORES, NPIX2 * C2)

    dummy = np.array([[0.25, 0.25, 0.75, 0.75]], np.float32)
    in_maps = []
    for k in range(NCORES):
        gsel = core_boxes[k]
        bk = boxes[gsel]
        sel_k = (box_ind[gsel] != k).astype(np.int64)
        npad = m_pad - bk.shape[0]
        if npad:
            bk = np.concatenate([bk, np.repeat(dummy, npad, 0)], axis=0)
            sel_k = np.concatenate([sel_k, np.zeros(npad, np.int64)])
        idx_w, w_all = _build_core_inputs(bk, sel_k, ng)
        in_maps.append({
            "img": np.concatenate([pairs[k], pairs[core_sec[k]]]),
            "idx": idx_w,
            "wts": w_all,
        })

    if ng not in _cache:
        _cache[ng] = _build_program(ng)
    nc = _cache[ng]

    res = bass_utils.run_bass_kernel_spmd(nc, in_maps,
                                          core_ids=list(range(NCORES)))
    global LAST_EXEC_NS
    LAST_EXEC_NS = res.exec_time_ns

    out = np.zeros((n_boxes, C, CROP, CROP), np.float32)
    for k in range(NCORES):
        nb = len(core_boxes[k])
        raw = np.asarray(res.results[k]["out"]).reshape(ng // UG, P, SU, C)
        ok = raw.transpose(0, 2, 1, 3).reshape(-1, C)[: nb * PX]
        ok = ok.reshape(nb, PX, C).transpose(0, 2, 1).astype(np.float32)
        out[core_boxes[k]] = ok.reshape(nb, C, CROP, CROP)
    return out


# revision 41
# speedup vs baseline: 1.1551x; 1.1551x over previous
"""CropAndResize (TF-style, crop 14x14) on 8 Trainium2 NeuronCores.

Strategy (data-parallel over ROIs, ~125 boxes per core):
  - Host: build a bf16 row-pair image per batch entry in channel-last
    layout: pairs[h, w] = (img[h, w, :], img[h+1, w, :]) -> [H-1, W, 2C].
    With this layout the 4 bilinear corners of one output pixel (rows
    ti/ti+1, cols xs/xs+1) are ONE contiguous 4C read (2 KB bf16), so a
    single SWDGE gather descriptor fetches a whole pixel's inputs.
  - Host: balance boxes across the 8 cores (each core = its own image
    plus at most one "secondary" donor image, concatenated in DRAM so
    int16 gather indices cover both), and precompute per-pixel corner
    weights w = [w_tl, w_bl, w_tr, w_br] * valid in f32.
  - Device: pipeline over units of 2 x 512-pixel gathers: per 128-pixel
    slot the weighted 4-corner sum runs on one of two paths chosen to
    balance the scalar(ACT) and vector(DVE) engines:
      path A (ACT-heavy): 4 ACT muls m_k = w_k * corner_k, then
        unit-batched DVE partial adds into the X/Y halves.
      path D (DVE-only): custom DVE op SCALE2_ADD computes
        x = w0*tl + w1*bl and y = w2*tr + w3*br (2 per-partition
        scalars, 2 tensor streams, one instruction each).
    ONE final TT per unit adds the X and Y halves for all 8 slots.
  - Host: scatter per-core outputs back to original box order, upcast.
"""

import numpy as np
import ml_dtypes

import concourse.bacc as bacc
import concourse.bass as bass
import concourse.tile as tile
from concourse import mybir, library_config, bass_utils

H, W, C = 100, 152, 256
CROP = 14
PX = CROP * CROP          # 196 pixels per box
P = 128                   # SBUF partitions
NCORES = 8
NPIX2 = (H - 1) * W       # 15048 pair-columns per image
C2 = 2 * C                # elems per pair-column (bf16)
C4 = 4 * C                # gather element: 2 adjacent pair-columns
GU = 512                  # indices per dma_gather instruction
SG = GU // P              # slots per gather group (4)
UG = 2                    # gather groups per compute unit
SU = UG * SG              # slots per compute unit (8)

F32 = mybir.dt.float32
BF16 = mybir.dt.bfloat16
I16 = mybir.dt.int16
ADD = mybir.AluOpType.add

_cache = {}
LAST_EXEC_NS = None

_S2A = None


def _register_s2a():
    """Runtime-register the custom DVE op out = in0*s0 + in1*s1."""
    global _S2A
    if _S2A is not None:
        return _S2A
    from concourse import dve_ops
    from concourse.dve_spec import Spec, Src0, Src1, C0, C1, lower
    from concourse.dve_uop import DveOpSpec
    from concourse.dve_table_gen import dve_ver_for

    name = "SCALE2_ADD_CR"
    for op in dve_ops.OPS:
        if op.name == name:
            _S2A = op
            return op
    ver = dve_ver_for("TRN2")
    spec = Spec(
        body=Src0 * C0 + Src1 * C1,
        reference=lambda in0, in1, s0, s1, imm2: (
            in0.astype(np.float32) * s0 + in1 * s1
        ),
    )
    dve_ops._SUB_OPCODE_FOR_NAME[name] = (
        max(dve_ops._SUB_OPCODE_FOR_NAME.values()) + 1
    )
    sha = DveOpSpec(
        name=name,
        opcode=dve_ops.get_dve_sub_opcode(name),
        uops=lower(spec, ver=ver),
        rd1_en=True,
    ).sha(ver)
    op = dve_ops.DveOp(name, spec, subdim=False, uops_sha={ver: sha})
    dve_ops.OPS.append(op)
    dve_ops.CUSTOM_DVE_SPECS[name] = spec
    _S2A = op
    return op


def _grid_params(boxes):
    """f32 mirror of the reference sampling grid -> effective pair-row
    start ts, pair-col start xs, and the 4 corner weights (valid-masked)."""
    f = np.float32
    y1, x1, y2, x2 = boxes[:, 0], boxes[:, 1], boxes[:, 2], boxes[:, 3]
    h_scale = (y2 - y1) * f(H - 1) / f(CROP - 1)
    w_scale = (x2 - x1) * f(W - 1) / f(CROP - 1)
    ar = np.arange(CROP, dtype=np.float32)
    in_y = y1[:, None] * f(H - 1) + ar[None, :] * h_scale[:, None]
    in_x = x1[:, None] * f(W - 1) + ar[None, :] * w_scale[:, None]
    valid_y = (in_y >= 0) & (in_y <= H - 1)
    valid_x = (in_x >= 0) & (in_x <= W - 1)
    top = np.floor(in_y)
    left = np.floor(in_x)
    y_lerp = (in_y - top).astype(np.float32)
    x_lerp = (in_x - left).astype(np.float32)
    ti = np.clip(top, 0, H - 1).astype(np.int32)
    bi = np.clip(top + 1, 0, H - 1).astype(np.int32)
    li = np.clip(left, 0, W - 1).astype(np.int32)
    ri = np.clip(left + 1, 0, W - 1).astype(np.int32)
    # pair-row start + effective lerps (ti==bi -> bottom row of the pair;
    # li==ri -> right col of the pair; invalid pixels are masked anyway)
    ts = np.minimum(ti, H - 2).astype(np.int32)
    yw = np.where(ti == bi, np.float32(1.0), y_lerp).astype(np.float32)
    xs = np.minimum(li, W - 2).astype(np.int32)
    xw = np.where(li == ri, np.float32(1.0), x_lerp).astype(np.float32)

    n = boxes.shape[0]
    yw2 = yw[:, :, None]
    xw2 = xw[:, None, :]
    vm = (valid_y[:, :, None] & valid_x[:, None, :]).astype(np.float32)
    w4 = np.empty((n, CROP, CROP, 4), np.float32)
    w4[..., 0] = (1 - yw2) * (1 - xw2) * vm   # tl
    w4[..., 1] = yw2 * (1 - xw2) * vm         # bl
    w4[..., 2] = (1 - yw2) * xw2 * vm         # tr
    w4[..., 3] = yw2 * xw2 * vm               # br
    idx = (ts[:, :, None] * W + xs[:, None, :]).reshape(n, PX)
    return idx, w4.reshape(n, PX, 4)


def _assign(box_ind):
    """Balance boxes so each core hosts <= L boxes from <= 2 images
    (its own + one donor). Returns per-core global box-index lists,
    per-core secondary image id, and L."""
    n = len(box_ind)
    counts = np.bincount(box_ind, minlength=NCORES)
    lists = [list(np.where(box_ind == k)[0]) for k in range(NCORES)]
    assign = {}
    L = int(counts.max())
    for L in range(-(-n // NCORES), int(counts.max()) + 1):
        donors = {k: int(counts[k]) - L for k in range(NCORES) if counts[k] > L}
        rooms = {k: L - int(counts[k]) for k in range(NCORES) if counts[k] < L}
        trial = {}
        ok = True
        for d, shed in sorted(donors.items(), key=lambda x: -x[1]):
            rem = shed
            for r, room in sorted(rooms.items(), key=lambda x: -x[1]):
                if rem <= 0:
                    break
                take = min(room, rem)
                if take > 0:
                    trial[r] = (d, take)
                    rem -= take
            for r in trial:
                rooms.pop(r, None)
            if rem > 0:
                ok = False
                break
        if ok:
            assign = trial
            break
    pulled = {}
    by_donor = {}
    for r, (d, take) in assign.items():
        by_donor.setdefault(d, []).append((r, take))
    for d, rts in by_donor.items():
        tail = lists[d]
        pos = len(tail)
        for r, take in rts:
            pulled[r] = tail[pos - take: pos]
            pos -= take
        lists[d] = tail[:pos]
    core_boxes, core_sec = [], []
    for k in range(NCORES):
        extra = pulled.get(k, [])
        sec = int(box_ind[extra[0]]) if extra else k
        core_boxes.append(list(lists[k]) + list(extra))
        core_sec.append(sec)
    return core_boxes, core_sec, L


def _build_core_inputs(boxes_k, sel_k, ng):
    """Gather indices (wrapped int16) + per-slot corner weights for one
    core. boxes_k: [m, 4]; sel_k: [m] in {0,1} (primary/secondary)."""
    ntot = ng * GU
    m = boxes_k.shape[0]
    idx, w4 = _grid_params(boxes_k)                      # [m,196], [m,196,4]
    idx = idx + (np.asarray(sel_k, np.int64)[:, None] * NPIX2)
    idx_flat = np.zeros(ntot, np.int16)
    idx_flat[: m * PX] = idx.reshape(-1).astype(np.int16)
    w_flat = np.zeros((ntot, 4), np.float32)
    w_flat[: m * PX] = w4.reshape(-1, 4)
    # wrapped idx layout: idx k at (k%16, k//16), tiled to 128 partitions
    wrapped = idx_flat.reshape(ntot // 16, 16).T         # [16, ntot//16]
    idx_w = np.tile(wrapped, (P // 16, 1))               # [128, ntot//16]
    # weights per (partition, slot, corner), f32
    w_ps = w_flat.reshape(ntot // P, P, 4).transpose(1, 0, 2)  # [P, slots, 4]
    w_all = np.ascontiguousarray(w_ps.reshape(P, -1))
    return idx_w, w_all


def _na_of(u):
    # alternate 3,3,4 ACT-path slots per unit (avg 10/24 of pixels on
    # the ACT path -- measured best balance of the two engines)
    return 4 if u % 3 == 2 else 3


def _build_program(ng):
    s2a = _register_s2a()
    nu = ng // UG
    nc = bacc.Bacc("TRN2", target_bir_lowering=False, debug=False,
                   num_devices=NCORES, num_swdge_queues=4)
    img = nc.dram_tensor("img", [2 * NPIX2 * C2], BF16, kind="ExternalInput")
    idx = nc.dram_tensor("idx", [P, ng * (GU // 16)], I16, kind="ExternalInput")
    wts = nc.dram_tensor("wts", [P, ng * SG * 4], F32, kind="ExternalInput")
    out = nc.dram_tensor("out", [ng * GU * C], BF16, kind="ExternalOutput")

    # overlapping gather view: index unit = one pair-column (C2 bf16),
    # payload = 2 adjacent pair-columns (C4 bf16 = 2 KB)
    gather_src = bass.AP(img, 0, [(C2, 2 * NPIX2 - 1), (1, C4)])

    with tile.TileContext(nc) as tc:
        with (
            tc.tile_pool(name="gat", bufs=3) as gat_pool,
            tc.tile_pool(name="osb", bufs=4) as out_pool,
            tc.tile_pool(name="meta", bufs=1) as meta_pool,
            tc.tile_pool(name="tmp", bufs=4) as tmp_pool,
        ):
            nc.gpsimd.load_library(library_config.mlp)
            idx_t = meta_pool.tile([P, ng * (GU // 16)], I16, tag="idx")
            nc.sync.dma_start(idx_t[:], idx[:])
            w_t = meta_pool.tile([P, ng * SG * 4], F32, tag="wts")
            nc.sync.dma_start(w_t[:], wts[:])

            LOOK = 2  # prefetch distance in units (2 gathers each)
            gts = {}

            def issue_gathers(u):
                gt = gat_pool.tile([P, UG, SG, C4], BF16, tag="g")
                for j in range(UG):
                    g = u * UG + j
                    nc.gpsimd.dma_gather(
                        gt[:, j], gather_src,
                        idx_t[:, g * (GU // 16): (g + 1) * (GU // 16)],
                        GU, GU, C4, elem_step=C2, queue_num=g % 4)
                gts[u] = gt

            for u in range(min(LOOK, nu)):
                issue_gathers(u)
            for u in range(nu):
                if u + LOOK < nu:
                    issue_gathers(u + LOOK)
                gt4 = gts.pop(u)

                def gs(s, lo, hi):
                    return gt4[:, s // SG, s % SG, lo:hi]

                NA = _na_of(u)
                ND = SU - NA
                ot = out_pool.tile([P, SU, C], BF16, tag="o")
                # X/Y halves: path A writes (m0+m1) into X, (m2+m3) into
                # Y; path D writes x/y directly; ONE final TT adds X+Y.
                xt = tmp_pool.tile([P, SU, C], BF16, tag="x")
                yt = tmp_pool.tile([P, SU, C], BF16, tag="y")

                # path A (ACT muls + batched partial adds): slots [0, NA)
                mt = [tmp_pool.tile([P, 4, C], BF16, tag=f"m{k}",
                                    name=f"mt{k}_{u}")
                      for k in range(4)]
                for a in range(NA):
                    sl = u * SU + a
                    for k in range(4):
                        nc.scalar.mul(
                            mt[k][:, a, :], gs(a, k * C, (k + 1) * C),
                            w_t[:, sl * 4 + k: sl * 4 + k + 1])
                nc.vector.tensor_tensor(xt[:, 0:NA, :], mt[0][:, 0:NA, :],
                                        mt[1][:, 0:NA, :], ADD)
                nc.vector.tensor_tensor(yt[:, 0:NA, :], mt[2][:, 0:NA, :],
                                        mt[3][:, 0:NA, :], ADD)

                # path D (custom dual-scale-add on DVE): slots [NA, SU)
                for d in range(ND):
                    s = NA + d
                    sl = u * SU + s
                    kb = sl * 4
                    nc.vector._custom_dve(
                        s2a, out=xt[:, s, :],
                        in0=gs(s, 0, C), in1=gs(s, C, 2 * C),
                        s0=w_t[:, kb: kb + 1], s1=w_t[:, kb + 1: kb + 2])
                    nc.vector._custom_dve(
                        s2a, out=yt[:, s, :],
                        in0=gs(s, 2 * C, 3 * C), in1=gs(s, 3 * C, 4 * C),
                        s0=w_t[:, kb + 2: kb + 3], s1=w_t[:, kb + 3: kb + 4])
                nc.vector.tensor_tensor(ot[:], xt[:], yt[:], ADD)

                # p-major store: each partition writes one contiguous
                # SU*C run; host unscrambles to pixel order
                out_ap = bass.AP(out, u * UG * GU * C,
                                 [(SU * C, P), (C, SU), (1, C)])
                nc.sync.dma_start(out_ap, ot[:])

    nc.compile()
    return nc


def kernel(image, boxes, box_ind):
    image = np.asarray(image, dtype=np.float32)
    boxes = np.asarray(boxes, dtype=np.float32)
    box_ind = np.asarray(box_ind)
    n_boxes = boxes.shape[0]

    core_boxes, core_sec, m_pad = _assign(box_ind)
    ng = -(-(m_pad * PX) // GU)
    ng = -(-ng // UG) * UG

    # channel-last bf16 row-pair images: pairs[k][h*W+w] = img[h,w,:]+img[h+1,w,:]
    image_t = image.transpose(0, 2, 3, 1).astype(ml_dtypes.bfloat16)  # [B,H,W,C]
    pairs = np.concatenate([image_t[:, :-1], image_t[:, 1:]], axis=-1)
    pairs = pairs.reshape(NCORES, NPIX2 * C2)

    dummy = np.array([[0.25, 0.25, 0.75, 0.75]], np.float32)
    in_maps = []
    for k in range(NCORES):
        gsel = core_boxes[k]
        bk = boxes[gsel]
        sel_k = (box_ind[gsel] != k).astype(np.int64)
        npad = m_pad - bk.shape[0]
        if npad:
            bk = np.concatenate([bk, np.repeat(dummy, npad, 0)], axis=0)
            sel_k = np.concatenate([sel_k, np.zeros(npad, np.int64)])
        idx_w, w_all = _build_core_inputs(bk, sel_k, ng)
        in_maps.append({
            "img": np.concatenate([pairs[k], pairs[core_sec[k]]]),
            "idx": idx_w,
            "wts": w_all,
        })

    if ng not in _cache:
        _cache[ng] = _build_program(ng)
    nc = _cache[ng]

    res = bass_utils.run_bass_kernel_spmd(nc, in_maps,
                                          core_ids=list(range(NCORES)))
    global LAST_EXEC_NS
    LAST_EXEC_NS = res.exec_time_ns

    out = np.zeros((n_boxes, C, CROP, CROP), np.float32)
    for k in range(NCORES):
        nb = len(core_boxes[k])
        raw = np.asarray(res.results[k]["out"]).reshape(ng // UG, P, SU, C)
        ok = raw.transpose(0, 2, 1, 3).reshape(-1, C)[: nb * PX]
        ok = ok.reshape(nb, PX, C).transpose(0, 2, 1).astype(np.float32)
        out[core_boxes[k]] = ok.reshape(nb, C, CROP, CROP)
    return out
